# revision 28
# baseline (speedup 1.0000x reference)
"""Ragged-sequence attention pooling on 8 TRN2 NeuronCores.

reference:
    scores[b,t] = sum_d seq[b,t,d] * cond[b,d]
    scores masked with -1e20 where t >= lens[b]
    out[b,:]   = softmax_t(scores) @ seq[b]   -> [B, D]

Sharding: data-parallel over B (32 batches -> 4 per core), with the
batch->core assignment balanced by sequence length (LPT bin packing).

Ragged packing: timesteps beyond lens[b] contribute nothing (softmax
weight exactly 0), so the host packs ONLY the valid 512-timestep blocks
of each sequence contiguously into a per-core buffer. The device runs a
static program over S_max slots (max packed-slab count over cores; pad
slots are zero-filled and fully masked). This roughly halves HBM
traffic vs reading all of seq — the problem's memory bottleneck.

Per-slot (block of 512 t = 4 tiles of 128 t, t on SBUF partitions):
  - DMA slab [128, 4, 1024] f32; ScalarE casts a bf16 copy for the PE.
  - DVE scalar_tensor_tensor: prod = slab*cond_bcast with fused row-sum
    -> scores[:, i]; additive mask; per-block max m_s via PE transpose.
  - ScalarE: p = exp(scores - m_s) (bf16) with fused row-sum l.
  - PE: 8 accumulating matmuls p^T @ slab_bf into PSUM row s
    ([S,512] x 2 banks), plus l_s = sum(p) via ones matmul.
Per-batch epilogue (all on tiny [1,S]/[S,1] tensors):
  w = exp(m + sel_mask - M_b); out_b = (w^T @ accs) / (w^T @ l).
"""

import numpy as np

import concourse.bacc as bacc
import concourse.bass as bass
import concourse.bass_isa as bass_isa
import concourse.tile as tile
from concourse import mybir
from concourse.bass_utils import run_bass_kernel_spmd

F32 = mybir.dt.float32
BF16 = mybir.dt.bfloat16
ALU = mybir.AluOpType
AF = mybir.ActivationFunctionType

B, T, D = 32, 4096, 1024
NCORES = 8
BPC = B // NCORES          # batches per core = 4
P = 128                    # partitions / timesteps per tile
BLK = 512                  # timesteps per slot (= 1 DMA slab)
SLAB = BLK // P            # 4 tiles per slot
NEG_INF = -1e20


def build_program(S):
    """S = number of packed slots each core processes (<= 32)."""
    assert S <= 32
    nc = bacc.Bacc("TRN2", target_bir_lowering=False, debug=False,
                   num_devices=NCORES)

    seqp = nc.dram_tensor("seqp", [S, P, SLAB, D], F32, kind="ExternalInput")
    condp = nc.dram_tensor("condp", [S, D], F32, kind="ExternalInput")
    maskp = nc.dram_tensor("maskp", [P, S, SLAB], F32, kind="ExternalInput")
    selm = nc.dram_tensor("selm", [BPC, S], F32, kind="ExternalInput")
    ident = nc.dram_tensor("ident", [BPC, BPC], F32, kind="ExternalInput")
    out = nc.dram_tensor("out", [BPC, D], F32, kind="ExternalOutput")

    with tile.TileContext(nc) as tc:
        with (
            tc.tile_pool(name="singles", bufs=1) as singles,
            tc.tile_pool(name="seqpool", bufs=5) as seqpool,
            tc.tile_pool(name="bfp", bufs=5) as bfp,
            tc.tile_pool(name="condpool", bufs=6) as condpool,
            tc.tile_pool(name="prodp", bufs=3) as prodp,
            tc.tile_pool(name="scorep", bufs=4) as scorep,
            tc.tile_pool(name="pexpp", bufs=4) as pexpp,
            tc.tile_pool(name="statp", bufs=4) as statp,
            tc.tile_pool(name="batchp", bufs=2) as batchp,
            tc.tile_pool(name="cstage", bufs=2) as cstage,
            tc.tile_pool(name="accp", bufs=4, space="PSUM") as accp,
            tc.tile_pool(name="miscp", bufs=2, space="PSUM") as miscp,
        ):
            # constants (setup DMAs go on the gpsimd SWDGE queue so the
            # sync HWDGE queue starts streaming seq immediately)
            ident_sb = singles.tile([BPC, BPC], F32)
            # tiny first DMA also absorbs the sync HWDGE ring warmup
            nc.sync.dma_start(out=ident_sb, in_=ident[:])
            mask_sb = singles.tile([P, S, SLAB], F32)
            nc.scalar.dma_start(out=mask_sb, in_=maskp[:])
            selm_sb = singles.tile([BPC, S], F32)
            nc.scalar.dma_start(out=selm_sb, in_=selm[:])
            ones_col = singles.tile([P, 1], F32)
            nc.vector.memset(ones_col, 1.0)
            ones_row = singles.tile([1, P], F32)
            nc.vector.memset(ones_row, 1.0)

            m_tab = singles.tile([P, S], F32)
            rowsums = singles.tile([P, S], F32)
            accs_sb = singles.tile([P, D], F32)
            # slots 0/1: cond via stride-0 broadcast DMA, issued before
            # the Q7 warm-up so slot 0 isn't gated by the Q7 IRAM load
            early_conds = []
            for s0 in range(min(4, S)):
                cond_t0 = condpool.tile([P, D], F32, tag="cond")
                cap = condp[s0:s0 + 1, :]
                nc.gpsimd.dma_start(
                    out=cond_t0,
                    in_=bass.AP(tensor=cap.tensor, offset=cap.offset,
                                ap=[[0, P]] + [list(x) for x in cap.ap]))
                early_conds.append(cond_t0)
            # pre-warm the Q7 partition ops (first call pays a ~6-9us
            # IRAM load; absorb it during the initial DMA fill). memset on
            # gpsimd so the warm-up has no cross-engine dependency.
            warm_in = statp.tile([P, 1], F32, tag="warm_in")
            nc.gpsimd.memset(warm_in, 1.0)
            warm = statp.tile([P, 1], F32, tag="warm")
            nc.gpsimd.partition_all_reduce(
                warm, warm_in, channels=P, reduce_op=bass_isa.ReduceOp.max)
            warm2 = statp.tile([P, 1], F32, tag="warm2")
            nc.gpsimd.partition_broadcast(warm2, warm_in[0:1, :], channels=P)

            for s in range(S):
                # per-slot cond: 4 KiB row DMA + on-chip broadcast
                if s < len(early_conds):
                    cond_t = early_conds[s]
                else:
                    crow = condpool.tile([1, D], F32, tag="crow")
                    nc.gpsimd.dma_start(out=crow, in_=condp[s:s + 1, :])
                    cond_t = condpool.tile([P, D], F32, tag="cond")
                    nc.gpsimd.partition_broadcast(cond_t, crow, channels=P)
                slab = seqpool.tile([P, SLAB, D], F32, tag="slab")
                if s < 2:
                    # per-tile DMAs so the first scores start ~4x sooner
                    for j in range(SLAB):
                        nc.sync.dma_start(out=slab[:, j, :],
                                          in_=seqp[s, :, j, :])
                else:
                    nc.sync.dma_start(out=slab, in_=seqp[s])
                slab_bf = bfp.tile([P, SLAB, D], BF16, tag="slabbf")
                nc.scalar.activation(slab_bf[:, 0:2, :], slab[:, 0:2, :],
                                     AF.Copy, scale=1.0)
                nc.scalar.activation(slab_bf[:, 2:4, :], slab[:, 2:4, :],
                                     AF.Copy, scale=1.0)

                scores = scorep.tile([P, SLAB], F32, tag="scores")
                for j in range(SLAB):
                    prod = prodp.tile([P, D], F32, tag="prod")
                    nc.vector.scalar_tensor_tensor(
                        out=prod, in0=slab[:, j, :], scalar=1.0,
                        in1=cond_t, op0=ALU.mult, op1=ALU.mult,
                        accum_out=scores[:, j:j + 1])
                nc.vector.tensor_add(scores, scores, mask_sb[:, s, :])
                rowmax = statp.tile([P, 1], F32, tag="rowmax")
                nc.vector.tensor_reduce(
                    out=rowmax, in_=scores, axis=mybir.AxisListType.X,
                    op=ALU.max)
                # block max on all partitions via the GpSimd daisy chain
                nc.gpsimd.partition_all_reduce(
                    m_tab[:, s:s + 1], rowmax, channels=P,
                    reduce_op=bass_isa.ReduceOp.max)
                negm = statp.tile([P, 1], F32, tag="negm")
                nc.scalar.activation(negm, m_tab[:, s:s + 1], AF.Copy,
                                     scale=-1.0)
                pexp = pexpp.tile([P, SLAB], BF16, tag="pexp")
                nc.scalar.activation(pexp, scores, AF.Exp, bias=negm,
                                     scale=1.0, accum_out=rowsums[:, s:s + 1])

                accA = accp.tile([1, 512], F32, tag="acc")
                accB = accp.tile([1, 512], F32, tag="acc")
                for j in range(SLAB):
                    st, sp = (j == 0), (j == SLAB - 1)
                    nc.tensor.matmul(
                        accA, lhsT=pexp[:, j:j + 1],
                        rhs=slab_bf[:, j, 0:512], start=st, stop=sp)
                    nc.tensor.matmul(
                        accB, lhsT=pexp[:, j:j + 1],
                        rhs=slab_bf[:, j, 512:1024], start=st, stop=sp)
                # park this slot's acc in row s of the SBUF accs table
                # (PSUM is not DMA-readable: stage via ScalarE first)
                apark = cstage.tile([1, D], F32, tag="apark")
                nc.scalar.activation(apark[:, 0:512], accA, AF.Copy)
                nc.scalar.activation(apark[:, 512:1024], accB, AF.Copy)
                nc.scalar.dma_start(out=accs_sb[s:s + 1, :], in_=apark)

            lcol_ps = miscp.tile([P, 1], F32, tag="misc")
            nc.tensor.matmul(lcol_ps[0:S, :], lhsT=rowsums, rhs=ones_col,
                             start=True, stop=True)
            lcol_sb = singles.tile([P, 1], F32)
            nc.scalar.activation(lcol_sb[0:S, :], lcol_ps[0:S, :], AF.Copy)

            # batched epilogue: all BPC batches on partitions 0..BPC-1
            # m4 = m_buf broadcast to BPC partitions (ones[1,BPC].T @ m_buf)
            m4_ps = miscp.tile([BPC, S], F32, tag="m4")
            nc.tensor.matmul(m4_ps, lhsT=ones_row[:, 0:BPC], rhs=m_tab[0:1, :],
                             start=True, stop=True)
            mrow4 = batchp.tile([BPC, S], F32, tag="mrow4")
            nc.vector.tensor_add(mrow4, m4_ps, selm_sb)
            Mx4 = batchp.tile([BPC, 1], F32, tag="Mx4")
            nc.vector.tensor_reduce(
                out=Mx4, in_=mrow4, axis=mybir.AxisListType.X, op=ALU.max)
            nMx4 = batchp.tile([BPC, 1], F32, tag="nMx4")
            nc.vector.tensor_scalar_mul(nMx4, Mx4, -1.0)
            wrow4 = batchp.tile([BPC, S], F32, tag="wrow4")
            nc.scalar.activation(wrow4, mrow4, AF.Exp, bias=nMx4, scale=1.0)
            # transpose to [S, BPC]: wrow4.T @ I_BPC
            wcol_ps = miscp.tile([P, BPC], F32, tag="misc")
            nc.tensor.matmul(wcol_ps[0:S, :], lhsT=wrow4,
                             rhs=ident_sb,
                             start=True, stop=True)
            wcol = batchp.tile([P, BPC], F32, tag="wcol")
            nc.scalar.activation(wcol[0:S, :], wcol_ps[0:S, :], AF.Copy)

            outpsA = accp.tile([BPC, 512], F32, tag="acc")
            outpsB = accp.tile([BPC, 512], F32, tag="acc")
            nc.tensor.matmul(outpsA, lhsT=wcol[0:S, :],
                             rhs=accs_sb[0:S, 0:512], start=True, stop=True)
            nc.tensor.matmul(outpsB, lhsT=wcol[0:S, :],
                             rhs=accs_sb[0:S, 512:1024],
                             start=True, stop=True)
            denps = miscp.tile([BPC, 1], F32, tag="misc")
            nc.tensor.matmul(denps, lhsT=wcol[0:S, :],
                             rhs=lcol_sb[0:S, :], start=True, stop=True)
            rden4 = batchp.tile([BPC, 1], F32, tag="rden4")
            nc.vector.reciprocal(rden4, denps)
            ostage = cstage.tile([BPC, D], F32, tag="ostage")
            nc.scalar.activation(ostage[:, 0:512], outpsA, AF.Copy,
                                 scale=rden4)
            nc.scalar.activation(ostage[:, 512:1024], outpsB, AF.Copy,
                                 scale=rden4)
            nc.sync.dma_start(out=out[:], in_=ostage)

    nc.compile()
    return nc


_NC_CACHE = {}


def _get_program(S):
    if S not in _NC_CACHE:
        _NC_CACHE[S] = build_program(S)
    return _NC_CACHE[S]


def plan_shards(lens):
    """LPT-balance batches to cores by packed-slab count (4 per core)."""
    lens = np.asarray(lens).astype(np.int64)
    nslab = np.maximum(1, -(-lens // BLK))  # ceil
    order = np.argsort(-nslab, kind="stable")
    loads = [0] * NCORES
    counts = [0] * NCORES
    assign = [[] for _ in range(NCORES)]
    for b in order:
        cands = [c for c in range(NCORES) if counts[c] < BPC]
        c = min(cands, key=lambda c: (loads[c], c))
        assign[c].append(int(b))
        loads[c] += int(nslab[b])
        counts[c] += 1
    S = max(loads)
    return assign, nslab, S


def make_in_maps(seq, lens, cond, assign, nslab, S):
    seq = np.asarray(seq)
    cond = np.asarray(cond)
    lens = np.asarray(lens).astype(np.int64)
    ident = np.eye(BPC, dtype=np.float32)
    in_maps = []
    for c in range(NCORES):
        seqp = np.zeros((S, P, SLAB, D), np.float32)
        condp = np.zeros((S, D), np.float32)
        maskp = np.full((P, S, SLAB), NEG_INF, np.float32)
        selm = np.full((BPC, S), NEG_INF, np.float32)
        cur = 0
        for bb, b in enumerate(assign[c]):
            k = int(nslab[b])
            blk = seq[b, :k * BLK].reshape(k, SLAB, P, D)
            seqp[cur:cur + k] = blk.transpose(0, 2, 1, 3)
            condp[cur:cur + k] = cond[b]
            # mask bias for slots cur..cur+k-1: t = kk*BLK + j*P + p
            t = (np.arange(k)[:, None, None] * BLK
                 + np.arange(SLAB)[None, :, None] * P
                 + np.arange(P)[None, None, :])  # [k, SLAB, P]
            mb = np.where(t < lens[b], 0.0, NEG_INF).astype(np.float32)
            maskp[:, cur:cur + k, :] = mb.transpose(2, 0, 1)
            selm[bb, cur:cur + k] = 0.0
            cur += k
        in_maps.append({
            "seqp": seqp,
            "condp": condp,
            "maskp": maskp,
            "selm": np.ascontiguousarray(selm),
            "ident": ident,
        })
    return in_maps


def run(seq, lens, cond, trace=False, **kw):
    assign, nslab, S = plan_shards(lens)
    nc = _get_program(S)
    in_maps = make_in_maps(seq, lens, cond, assign, nslab, S)
    res = run_bass_kernel_spmd(nc, in_maps, core_ids=list(range(NCORES)),
                               trace=trace, **kw)
    outs = np.zeros((B, D), np.float32)
    for c in range(NCORES):
        for bb, b in enumerate(assign[c]):
            outs[b] = res.results[c]["out"][bb]
    return outs, res


def kernel(seq, lens, cond):
    outs, _ = run(seq, lens, cond)
    return outs


# revision 30
# speedup vs baseline: 1.0485x; 1.0485x over previous
"""Ragged-sequence attention pooling on 8 TRN2 NeuronCores.

reference:
    scores[b,t] = sum_d seq[b,t,d] * cond[b,d]
    scores masked with -1e20 where t >= lens[b]
    out[b,:]   = softmax_t(scores) @ seq[b]   -> [B, D]

Sharding: data-parallel over B (32 batches -> 4 per core), with the
batch->core assignment balanced by sequence length (LPT bin packing).

Ragged packing: timesteps beyond lens[b] contribute nothing (softmax
weight exactly 0), so the host packs ONLY the valid 512-timestep blocks
of each sequence contiguously into a per-core buffer. The device runs a
static program over S_max slots (max packed-slab count over cores; pad
slots are zero-filled and fully masked). This roughly halves HBM
traffic vs reading all of seq — the problem's memory bottleneck.

Per-slot (block of 512 t = 4 tiles of 128 t, t on SBUF partitions):
  - DMA slab [128, 4, 1024] f32; ScalarE casts a bf16 copy for the PE.
  - DVE scalar_tensor_tensor: prod = slab*cond_bcast with fused row-sum
    -> scores[:, i]; additive mask; per-block max m_s via PE transpose.
  - ScalarE: p = exp(scores - m_s) (bf16) with fused row-sum l.
  - PE: 8 accumulating matmuls p^T @ slab_bf into PSUM row s
    ([S,512] x 2 banks), plus l_s = sum(p) via ones matmul.
Per-batch epilogue (all on tiny [1,S]/[S,1] tensors):
  w = exp(m + sel_mask - M_b); out_b = (w^T @ accs) / (w^T @ l).
"""

import numpy as np

import concourse.bacc as bacc
import concourse.bass as bass
import concourse.bass_isa as bass_isa
import concourse.tile as tile
from concourse import mybir
from concourse.bass_utils import run_bass_kernel_spmd

F32 = mybir.dt.float32
BF16 = mybir.dt.bfloat16
ALU = mybir.AluOpType
AF = mybir.ActivationFunctionType

B, T, D = 32, 4096, 1024
NCORES = 8
BPC = B // NCORES          # batches per core = 4
P = 128                    # partitions / timesteps per tile
BLK = 512                  # timesteps per slot (= 1 DMA slab)
SLAB = BLK // P            # 4 tiles per slot
NEG_INF = -1e20


def build_program(S):
    """S = number of packed slots each core processes (<= 32)."""
    assert S <= 32
    nc = bacc.Bacc("TRN2", target_bir_lowering=False, debug=False,
                   num_devices=NCORES)

    seqp = nc.dram_tensor("seqp", [S, P, SLAB, D], F32, kind="ExternalInput")
    cond = nc.dram_tensor("cond", [BPC, D], F32, kind="ExternalInput")
    bidx = nc.dram_tensor("bidx", [1, S], mybir.dt.int32, kind="ExternalInput")
    maskp = nc.dram_tensor("maskp", [P, S, SLAB], F32, kind="ExternalInput")
    selm = nc.dram_tensor("selm", [BPC, S], F32, kind="ExternalInput")
    ident = nc.dram_tensor("ident", [BPC, BPC], F32, kind="ExternalInput")
    out = nc.dram_tensor("out", [BPC, D], F32, kind="ExternalOutput")

    with tile.TileContext(nc) as tc:
        with (
            tc.tile_pool(name="singles", bufs=1) as singles,
            tc.tile_pool(name="seqpool", bufs=5) as seqpool,
            tc.tile_pool(name="bfp", bufs=5) as bfp,
            tc.tile_pool(name="prodp", bufs=3) as prodp,
            tc.tile_pool(name="scorep", bufs=4) as scorep,
            tc.tile_pool(name="pexpp", bufs=4) as pexpp,
            tc.tile_pool(name="statp", bufs=4) as statp,
            tc.tile_pool(name="batchp", bufs=2) as batchp,
            tc.tile_pool(name="cstage", bufs=2) as cstage,
            tc.tile_pool(name="accp", bufs=4, space="PSUM") as accp,
            tc.tile_pool(name="miscp", bufs=2, space="PSUM") as miscp,
        ):
            # constants (setup DMAs go on the gpsimd SWDGE queue so the
            # sync HWDGE queue starts streaming seq immediately)
            ident_sb = singles.tile([BPC, BPC], F32)
            # tiny first DMA also absorbs the sync HWDGE ring warmup
            nc.sync.dma_start(out=ident_sb, in_=ident[:])
            mask_sb = singles.tile([P, S, SLAB], F32)
            nc.scalar.dma_start(out=mask_sb, in_=maskp[:])
            selm_sb = singles.tile([BPC, S], F32)
            nc.scalar.dma_start(out=selm_sb, in_=selm[:])
            ones_col = singles.tile([P, 1], F32)
            nc.vector.memset(ones_col, 1.0)
            ones_row = singles.tile([1, P], F32)
            nc.vector.memset(ones_row, 1.0)

            m_tab = singles.tile([P, S], F32)
            rowsums = singles.tile([P, S], F32)
            accs_sb = singles.tile([P, D], F32)
            # per-batch cond rows broadcast to all partitions once;
            # slots select theirs via a runtime register offset
            cond_all = singles.tile([P, BPC * D], F32)
            for bb in range(BPC):
                cap = cond[bb:bb + 1, :]
                nc.gpsimd.dma_start(
                    out=cond_all[:, bb * D:(bb + 1) * D],
                    in_=bass.AP(tensor=cap.tensor, offset=cap.offset,
                                ap=[[0, P]] + [list(x) for x in cap.ap]))
            bidx_sb = singles.tile([1, S], mybir.dt.int32)
            nc.scalar.dma_start(out=bidx_sb, in_=bidx[:])
            # pre-warm the Q7 partition_all_reduce (first call pays a
            # ~6-9us IRAM load; absorb it during the initial DMA fill)
            warm_in = statp.tile([P, 1], F32, tag="warm_in")
            nc.gpsimd.memset(warm_in, 1.0)
            warm = statp.tile([P, 1], F32, tag="warm")
            nc.gpsimd.partition_all_reduce(
                warm, warm_in, channels=P, reduce_op=bass_isa.ReduceOp.max)

            for s in range(S):
                # select this slot's cond via runtime offset (bb*D)
                breg = nc.alloc_registers(engines=[mybir.EngineType.DVE])
                nc.regs_load(breg, bidx_sb[:, s:s + 1])
                boff = nc.snap(breg, donate=True, min_val=0,
                               max_val=(BPC - 1) * D)
                cond_t = cond_all[:, bass.ds(boff, D)]
                slab = seqpool.tile([P, SLAB, D], F32, tag="slab")
                if s < 2:
                    # per-tile DMAs so the first scores start ~4x sooner
                    for j in range(SLAB):
                        nc.sync.dma_start(out=slab[:, j, :],
                                          in_=seqp[s, :, j, :])
                else:
                    nc.sync.dma_start(out=slab, in_=seqp[s])
                slab_bf = bfp.tile([P, SLAB, D], BF16, tag="slabbf")
                nc.scalar.activation(slab_bf[:, 0:2, :], slab[:, 0:2, :],
                                     AF.Copy, scale=1.0)
                nc.scalar.activation(slab_bf[:, 2:4, :], slab[:, 2:4, :],
                                     AF.Copy, scale=1.0)

                scores = scorep.tile([P, SLAB], F32, tag="scores")
                for j in range(SLAB):
                    prod = prodp.tile([P, D], F32, tag="prod")
                    nc.vector.scalar_tensor_tensor(
                        out=prod, in0=slab[:, j, :], scalar=1.0,
                        in1=cond_t, op0=ALU.mult, op1=ALU.mult,
                        accum_out=scores[:, j:j + 1])
                nc.vector.tensor_add(scores, scores, mask_sb[:, s, :])
                rowmax = statp.tile([P, 1], F32, tag="rowmax")
                nc.vector.tensor_reduce(
                    out=rowmax, in_=scores, axis=mybir.AxisListType.X,
                    op=ALU.max)
                # block max on all partitions via the GpSimd daisy chain
                nc.gpsimd.partition_all_reduce(
                    m_tab[:, s:s + 1], rowmax, channels=P,
                    reduce_op=bass_isa.ReduceOp.max)
                negm = statp.tile([P, 1], F32, tag="negm")
                nc.scalar.activation(negm, m_tab[:, s:s + 1], AF.Copy,
                                     scale=-1.0)
                pexp = pexpp.tile([P, SLAB], BF16, tag="pexp")
                nc.scalar.activation(pexp, scores, AF.Exp, bias=negm,
                                     scale=1.0, accum_out=rowsums[:, s:s + 1])

                accA = accp.tile([1, 512], F32, tag="acc")
                accB = accp.tile([1, 512], F32, tag="acc")
                for j in range(SLAB):
                    st, sp = (j == 0), (j == SLAB - 1)
                    nc.tensor.matmul(
                        accA, lhsT=pexp[:, j:j + 1],
                        rhs=slab_bf[:, j, 0:512], start=st, stop=sp)
                    nc.tensor.matmul(
                        accB, lhsT=pexp[:, j:j + 1],
                        rhs=slab_bf[:, j, 512:1024], start=st, stop=sp)
                # park this slot's acc in row s of the SBUF accs table
                # (PSUM is not DMA-readable: stage via ScalarE first)
                apark = cstage.tile([1, D], F32, tag="apark")
                nc.scalar.activation(apark[:, 0:512], accA, AF.Copy)
                nc.scalar.activation(apark[:, 512:1024], accB, AF.Copy)
                nc.scalar.dma_start(out=accs_sb[s:s + 1, :], in_=apark)

            lcol_ps = miscp.tile([P, 1], F32, tag="misc")
            nc.tensor.matmul(lcol_ps[0:S, :], lhsT=rowsums, rhs=ones_col,
                             start=True, stop=True)
            lcol_sb = singles.tile([P, 1], F32)
            nc.scalar.activation(lcol_sb[0:S, :], lcol_ps[0:S, :], AF.Copy)

            # batched epilogue: all BPC batches on partitions 0..BPC-1
            # m4 = m_buf broadcast to BPC partitions (ones[1,BPC].T @ m_buf)
            m4_ps = miscp.tile([BPC, S], F32, tag="m4")
            nc.tensor.matmul(m4_ps, lhsT=ones_row[:, 0:BPC], rhs=m_tab[0:1, :],
                             start=True, stop=True)
            mrow4 = batchp.tile([BPC, S], F32, tag="mrow4")
            nc.vector.tensor_add(mrow4, m4_ps, selm_sb)
            Mx4 = batchp.tile([BPC, 1], F32, tag="Mx4")
            nc.vector.tensor_reduce(
                out=Mx4, in_=mrow4, axis=mybir.AxisListType.X, op=ALU.max)
            nMx4 = batchp.tile([BPC, 1], F32, tag="nMx4")
            nc.vector.tensor_scalar_mul(nMx4, Mx4, -1.0)
            wrow4 = batchp.tile([BPC, S], F32, tag="wrow4")
            nc.scalar.activation(wrow4, mrow4, AF.Exp, bias=nMx4, scale=1.0)
            # transpose to [S, BPC]: wrow4.T @ I_BPC
            wcol_ps = miscp.tile([P, BPC], F32, tag="misc")
            nc.tensor.matmul(wcol_ps[0:S, :], lhsT=wrow4,
                             rhs=ident_sb,
                             start=True, stop=True)
            wcol = batchp.tile([P, BPC], F32, tag="wcol")
            nc.scalar.activation(wcol[0:S, :], wcol_ps[0:S, :], AF.Copy)

            outpsA = accp.tile([BPC, 512], F32, tag="acc")
            outpsB = accp.tile([BPC, 512], F32, tag="acc")
            nc.tensor.matmul(outpsA, lhsT=wcol[0:S, :],
                             rhs=accs_sb[0:S, 0:512], start=True, stop=True)
            nc.tensor.matmul(outpsB, lhsT=wcol[0:S, :],
                             rhs=accs_sb[0:S, 512:1024],
                             start=True, stop=True)
            denps = miscp.tile([BPC, 1], F32, tag="misc")
            nc.tensor.matmul(denps, lhsT=wcol[0:S, :],
                             rhs=lcol_sb[0:S, :], start=True, stop=True)
            rden4 = batchp.tile([BPC, 1], F32, tag="rden4")
            nc.vector.reciprocal(rden4, denps)
            ostage = cstage.tile([BPC, D], F32, tag="ostage")
            nc.scalar.activation(ostage[:, 0:512], outpsA, AF.Copy,
                                 scale=rden4)
            nc.scalar.activation(ostage[:, 512:1024], outpsB, AF.Copy,
                                 scale=rden4)
            nc.sync.dma_start(out=out[:], in_=ostage)

    nc.compile()
    return nc


_NC_CACHE = {}


def _get_program(S):
    if S not in _NC_CACHE:
        _NC_CACHE[S] = build_program(S)
    return _NC_CACHE[S]


def plan_shards(lens):
    """LPT-balance batches to cores by packed-slab count (4 per core)."""
    lens = np.asarray(lens).astype(np.int64)
    nslab = np.maximum(1, -(-lens // BLK))  # ceil
    order = np.argsort(-nslab, kind="stable")
    loads = [0] * NCORES
    counts = [0] * NCORES
    assign = [[] for _ in range(NCORES)]
    for b in order:
        cands = [c for c in range(NCORES) if counts[c] < BPC]
        c = min(cands, key=lambda c: (loads[c], c))
        assign[c].append(int(b))
        loads[c] += int(nslab[b])
        counts[c] += 1
    S = max(loads)
    return assign, nslab, S


def make_in_maps(seq, lens, cond, assign, nslab, S):
    seq = np.asarray(seq)
    cond = np.asarray(cond)
    lens = np.asarray(lens).astype(np.int64)
    ident = np.eye(BPC, dtype=np.float32)
    in_maps = []
    for c in range(NCORES):
        seqp = np.zeros((S, P, SLAB, D), np.float32)
        condc = np.zeros((BPC, D), np.float32)
        bidx = np.zeros((1, S), np.int32)
        maskp = np.full((P, S, SLAB), NEG_INF, np.float32)
        selm = np.full((BPC, S), NEG_INF, np.float32)
        cur = 0
        for bb, b in enumerate(assign[c]):
            k = int(nslab[b])
            blk = seq[b, :k * BLK].reshape(k, SLAB, P, D)
            seqp[cur:cur + k] = blk.transpose(0, 2, 1, 3)
            condc[bb] = cond[b]
            bidx[0, cur:cur + k] = bb * D
            # mask bias for slots cur..cur+k-1: t = kk*BLK + j*P + p
            t = (np.arange(k)[:, None, None] * BLK
                 + np.arange(SLAB)[None, :, None] * P
                 + np.arange(P)[None, None, :])  # [k, SLAB, P]
            mb = np.where(t < lens[b], 0.0, NEG_INF).astype(np.float32)
            maskp[:, cur:cur + k, :] = mb.transpose(2, 0, 1)
            selm[bb, cur:cur + k] = 0.0
            cur += k
        in_maps.append({
            "seqp": seqp,
            "cond": condc,
            "bidx": bidx,
            "maskp": maskp,
            "selm": np.ascontiguousarray(selm),
            "ident": ident,
        })
    return in_maps


def run(seq, lens, cond, trace=False, **kw):
    assign, nslab, S = plan_shards(lens)
    nc = _get_program(S)
    in_maps = make_in_maps(seq, lens, cond, assign, nslab, S)
    res = run_bass_kernel_spmd(nc, in_maps, core_ids=list(range(NCORES)),
                               trace=trace, **kw)
    outs = np.zeros((B, D), np.float32)
    for c in range(NCORES):
        for bb, b in enumerate(assign[c]):
            outs[b] = res.results[c]["out"][bb]
    return outs, res


def kernel(seq, lens, cond):
    outs, _ = run(seq, lens, cond)
    return outs


# revision 31
# speedup vs baseline: 1.0680x; 1.0186x over previous
"""Ragged-sequence attention pooling on 8 TRN2 NeuronCores.

reference:
    scores[b,t] = sum_d seq[b,t,d] * cond[b,d]
    scores masked with -1e20 where t >= lens[b]
    out[b,:]   = softmax_t(scores) @ seq[b]   -> [B, D]

Sharding: data-parallel over B (32 batches -> 4 per core), with the
batch->core assignment balanced by sequence length (LPT bin packing).

Ragged packing: timesteps beyond lens[b] contribute nothing (softmax
weight exactly 0), so the host packs ONLY the valid 512-timestep blocks
of each sequence contiguously into a per-core buffer. The device runs a
static program over S_max slots (max packed-slab count over cores; pad
slots are zero-filled and fully masked). This roughly halves HBM
traffic vs reading all of seq — the problem's memory bottleneck.

Per-slot (block of 512 t = 4 tiles of 128 t, t on SBUF partitions):
  - DMA slab [128, 4, 1024] f32; ScalarE casts a bf16 copy for the PE.
  - DVE scalar_tensor_tensor: prod = slab*cond_bcast with fused row-sum
    -> scores[:, i]; additive mask; per-block max m_s via PE transpose.
  - ScalarE: p = exp(scores - m_s) (bf16) with fused row-sum l.
  - PE: 8 accumulating matmuls p^T @ slab_bf into PSUM row s
    ([S,512] x 2 banks), plus l_s = sum(p) via ones matmul.
Per-batch epilogue (all on tiny [1,S]/[S,1] tensors):
  w = exp(m + sel_mask - M_b); out_b = (w^T @ accs) / (w^T @ l).
"""

import numpy as np

import concourse.bacc as bacc
import concourse.bass as bass
import concourse.bass_isa as bass_isa
import concourse.tile as tile
from concourse import mybir
from concourse.bass_utils import run_bass_kernel_spmd

F32 = mybir.dt.float32
BF16 = mybir.dt.bfloat16
ALU = mybir.AluOpType
AF = mybir.ActivationFunctionType

B, T, D = 32, 4096, 1024
NCORES = 8
BPC = B // NCORES          # batches per core = 4
P = 128                    # partitions / timesteps per tile
BLK = 512                  # timesteps per slot (= 1 DMA slab)
SLAB = BLK // P            # 4 tiles per slot
NEG_INF = -1e20


def build_program(S):
    """S = number of packed slots each core processes (<= 32)."""
    assert S <= 32
    nc = bacc.Bacc("TRN2", target_bir_lowering=False, debug=False,
                   num_devices=NCORES)

    seqp = nc.dram_tensor("seqp", [S, P, SLAB, D], F32, kind="ExternalInput")
    cond = nc.dram_tensor("cond", [BPC, D], F32, kind="ExternalInput")
    bidx = nc.dram_tensor("bidx", [1, S], mybir.dt.int32, kind="ExternalInput")
    maskp = nc.dram_tensor("maskp", [P, S, SLAB], F32, kind="ExternalInput")
    selm = nc.dram_tensor("selm", [BPC, S], F32, kind="ExternalInput")
    ident = nc.dram_tensor("ident", [BPC, BPC], F32, kind="ExternalInput")
    out = nc.dram_tensor("out", [BPC, D], F32, kind="ExternalOutput")

    with tile.TileContext(nc) as tc:
        with (
            tc.tile_pool(name="singles", bufs=1) as singles,
            tc.tile_pool(name="seqpool", bufs=5) as seqpool,
            tc.tile_pool(name="bfp", bufs=5) as bfp,
            tc.tile_pool(name="prodp", bufs=3) as prodp,
            tc.tile_pool(name="scorep", bufs=4) as scorep,
            tc.tile_pool(name="pexpp", bufs=4) as pexpp,
            tc.tile_pool(name="statp", bufs=4) as statp,
            tc.tile_pool(name="batchp", bufs=2) as batchp,
            tc.tile_pool(name="cstage", bufs=2) as cstage,
            tc.tile_pool(name="accp", bufs=4, space="PSUM") as accp,
            tc.tile_pool(name="miscp", bufs=2, space="PSUM") as miscp,
        ):
            # per-batch cond rows broadcast to all partitions (first on
            # the gpsimd queue: the first score op waits on all four) and
            # the slot->batch offset table (first on the scalar queue)
            cond_all = singles.tile([P, BPC * D], F32)
            for bb in range(BPC):
                cap = cond[bb:bb + 1, :]
                nc.gpsimd.dma_start(
                    out=cond_all[:, bb * D:(bb + 1) * D],
                    in_=bass.AP(tensor=cap.tensor, offset=cap.offset,
                                ap=[[0, P]] + [list(x) for x in cap.ap]))
            bidx_sb = singles.tile([1, S], mybir.dt.int32)
            nc.scalar.dma_start(out=bidx_sb, in_=bidx[:])
            ident_sb = singles.tile([BPC, BPC], F32)
            # tiny first DMA also absorbs the sync HWDGE ring warmup
            nc.sync.dma_start(out=ident_sb, in_=ident[:])
            mask_sb = singles.tile([P, S, SLAB], F32)
            nc.scalar.dma_start(out=mask_sb, in_=maskp[:])
            selm_sb = singles.tile([BPC, S], F32)
            nc.scalar.dma_start(out=selm_sb, in_=selm[:])
            ones_col = singles.tile([P, 1], F32)
            nc.vector.memset(ones_col, 1.0)
            ones_row = singles.tile([1, P], F32)
            nc.vector.memset(ones_row, 1.0)

            m_tab = singles.tile([P, S], F32)
            rowsums = singles.tile([P, S], F32)
            accs_sb = singles.tile([P, D], F32)
            # pre-warm the Q7 partition_all_reduce (first call pays a
            # ~6-9us IRAM load; absorb it during the initial DMA fill)
            warm_in = statp.tile([P, 1], F32, tag="warm_in")
            nc.gpsimd.memset(warm_in, 1.0)
            warm = statp.tile([P, 1], F32, tag="warm")
            nc.gpsimd.partition_all_reduce(
                warm, warm_in, channels=P, reduce_op=bass_isa.ReduceOp.max)

            for s in range(S):
                # select this slot's cond via runtime offset (bb*D)
                breg = nc.alloc_registers(engines=[mybir.EngineType.DVE])
                nc.regs_load(breg, bidx_sb[:, s:s + 1])
                boff = nc.snap(breg, donate=True, min_val=0,
                               max_val=(BPC - 1) * D)
                cond_t = cond_all[:, bass.ds(boff, D)]
                slab = seqpool.tile([P, SLAB, D], F32, tag="slab")
                if s < 2:
                    # per-tile DMAs so the first scores start ~4x sooner
                    for j in range(SLAB):
                        nc.sync.dma_start(out=slab[:, j, :],
                                          in_=seqp[s, :, j, :])
                else:
                    nc.sync.dma_start(out=slab, in_=seqp[s])
                slab_bf = bfp.tile([P, SLAB, D], BF16, tag="slabbf")
                nc.scalar.activation(slab_bf[:, 0:2, :], slab[:, 0:2, :],
                                     AF.Copy, scale=1.0)
                nc.scalar.activation(slab_bf[:, 2:4, :], slab[:, 2:4, :],
                                     AF.Copy, scale=1.0)

                scores = scorep.tile([P, SLAB], F32, tag="scores")
                for j in range(SLAB):
                    prod = prodp.tile([P, D], F32, tag="prod")
                    nc.vector.scalar_tensor_tensor(
                        out=prod, in0=slab[:, j, :], scalar=1.0,
                        in1=cond_t, op0=ALU.mult, op1=ALU.mult,
                        accum_out=scores[:, j:j + 1])
                nc.vector.tensor_add(scores, scores, mask_sb[:, s, :])
                rowmax = statp.tile([P, 1], F32, tag="rowmax")
                nc.vector.tensor_reduce(
                    out=rowmax, in_=scores, axis=mybir.AxisListType.X,
                    op=ALU.max)
                # block max on all partitions via the GpSimd daisy chain
                nc.gpsimd.partition_all_reduce(
                    m_tab[:, s:s + 1], rowmax, channels=P,
                    reduce_op=bass_isa.ReduceOp.max)
                negm = statp.tile([P, 1], F32, tag="negm")
                nc.vector.tensor_scalar_mul(negm, m_tab[:, s:s + 1], -1.0)
                pexp = pexpp.tile([P, SLAB], BF16, tag="pexp")
                nc.scalar.activation(pexp, scores, AF.Exp, bias=negm,
                                     scale=1.0, accum_out=rowsums[:, s:s + 1])

                accA = accp.tile([1, 512], F32, tag="acc")
                accB = accp.tile([1, 512], F32, tag="acc")
                for j in range(SLAB):
                    st, sp = (j == 0), (j == SLAB - 1)
                    nc.tensor.matmul(
                        accA, lhsT=pexp[:, j:j + 1],
                        rhs=slab_bf[:, j, 0:512], start=st, stop=sp)
                    nc.tensor.matmul(
                        accB, lhsT=pexp[:, j:j + 1],
                        rhs=slab_bf[:, j, 512:1024], start=st, stop=sp)
                # park this slot's acc in row s of the SBUF accs table
                # (PSUM is not DMA-readable: stage via ScalarE first)
                apark = cstage.tile([1, D], F32, tag="apark")
                nc.scalar.activation(apark[:, 0:512], accA, AF.Copy)
                nc.scalar.activation(apark[:, 512:1024], accB, AF.Copy)
                nc.gpsimd.dma_start(out=accs_sb[s:s + 1, :], in_=apark)

            lcol_ps = miscp.tile([P, 1], F32, tag="misc")
            nc.tensor.matmul(lcol_ps[0:S, :], lhsT=rowsums, rhs=ones_col,
                             start=True, stop=True)
            lcol_sb = singles.tile([P, 1], F32)
            nc.scalar.activation(lcol_sb[0:S, :], lcol_ps[0:S, :], AF.Copy)

            # batched epilogue: all BPC batches on partitions 0..BPC-1
            # m4 = m_buf broadcast to BPC partitions (ones[1,BPC].T @ m_buf)
            m4_ps = miscp.tile([BPC, S], F32, tag="m4")
            nc.tensor.matmul(m4_ps, lhsT=ones_row[:, 0:BPC], rhs=m_tab[0:1, :],
                             start=True, stop=True)
            mrow4 = batchp.tile([BPC, S], F32, tag="mrow4")
            nc.vector.tensor_add(mrow4, m4_ps, selm_sb)
            Mx4 = batchp.tile([BPC, 1], F32, tag="Mx4")
            nc.vector.tensor_reduce(
                out=Mx4, in_=mrow4, axis=mybir.AxisListType.X, op=ALU.max)
            nMx4 = batchp.tile([BPC, 1], F32, tag="nMx4")
            nc.vector.tensor_scalar_mul(nMx4, Mx4, -1.0)
            wrow4 = batchp.tile([BPC, S], F32, tag="wrow4")
            nc.scalar.activation(wrow4, mrow4, AF.Exp, bias=nMx4, scale=1.0)
            # transpose to [S, BPC]: wrow4.T @ I_BPC
            wcol_ps = miscp.tile([P, BPC], F32, tag="misc")
            nc.tensor.matmul(wcol_ps[0:S, :], lhsT=wrow4,
                             rhs=ident_sb,
                             start=True, stop=True)
            wcol = batchp.tile([P, BPC], F32, tag="wcol")
            nc.scalar.activation(wcol[0:S, :], wcol_ps[0:S, :], AF.Copy)

            outpsA = accp.tile([BPC, 512], F32, tag="acc")
            outpsB = accp.tile([BPC, 512], F32, tag="acc")
            nc.tensor.matmul(outpsA, lhsT=wcol[0:S, :],
                             rhs=accs_sb[0:S, 0:512], start=True, stop=True)
            nc.tensor.matmul(outpsB, lhsT=wcol[0:S, :],
                             rhs=accs_sb[0:S, 512:1024],
                             start=True, stop=True)
            denps = miscp.tile([BPC, 1], F32, tag="misc")
            nc.tensor.matmul(denps, lhsT=wcol[0:S, :],
                             rhs=lcol_sb[0:S, :], start=True, stop=True)
            rden4 = batchp.tile([BPC, 1], F32, tag="rden4")
            nc.vector.reciprocal(rden4, denps)
            ostage = cstage.tile([BPC, D], F32, tag="ostage")
            nc.scalar.activation(ostage[:, 0:512], outpsA, AF.Copy,
                                 scale=rden4)
            nc.scalar.activation(ostage[:, 512:1024], outpsB, AF.Copy,
                                 scale=rden4)
            nc.sync.dma_start(out=out[:], in_=ostage)

    nc.compile()
    return nc


_NC_CACHE = {}


def _get_program(S):
    if S not in _NC_CACHE:
        _NC_CACHE[S] = build_program(S)
    return _NC_CACHE[S]


def plan_shards(lens):
    """LPT-balance batches to cores by packed-slab count (4 per core)."""
    lens = np.asarray(lens).astype(np.int64)
    nslab = np.maximum(1, -(-lens // BLK))  # ceil
    order = np.argsort(-nslab, kind="stable")
    loads = [0] * NCORES
    counts = [0] * NCORES
    assign = [[] for _ in range(NCORES)]
    for b in order:
        cands = [c for c in range(NCORES) if counts[c] < BPC]
        c = min(cands, key=lambda c: (loads[c], c))
        assign[c].append(int(b))
        loads[c] += int(nslab[b])
        counts[c] += 1
    S = max(loads)
    return assign, nslab, S


def make_in_maps(seq, lens, cond, assign, nslab, S):
    seq = np.asarray(seq)
    cond = np.asarray(cond)
    lens = np.asarray(lens).astype(np.int64)
    ident = np.eye(BPC, dtype=np.float32)
    in_maps = []
    for c in range(NCORES):
        seqp = np.zeros((S, P, SLAB, D), np.float32)
        condc = np.zeros((BPC, D), np.float32)
        bidx = np.zeros((1, S), np.int32)
        maskp = np.full((P, S, SLAB), NEG_INF, np.float32)
        selm = np.full((BPC, S), NEG_INF, np.float32)
        cur = 0
        for bb, b in enumerate(assign[c]):
            k = int(nslab[b])
            blk = seq[b, :k * BLK].reshape(k, SLAB, P, D)
            seqp[cur:cur + k] = blk.transpose(0, 2, 1, 3)
            condc[bb] = cond[b]
            bidx[0, cur:cur + k] = bb * D
            # mask bias for slots cur..cur+k-1: t = kk*BLK + j*P + p
            t = (np.arange(k)[:, None, None] * BLK
                 + np.arange(SLAB)[None, :, None] * P
                 + np.arange(P)[None, None, :])  # [k, SLAB, P]
            mb = np.where(t < lens[b], 0.0, NEG_INF).astype(np.float32)
            maskp[:, cur:cur + k, :] = mb.transpose(2, 0, 1)
            selm[bb, cur:cur + k] = 0.0
            cur += k
        in_maps.append({
            "seqp": seqp,
            "cond": condc,
            "bidx": bidx,
            "maskp": maskp,
            "selm": np.ascontiguousarray(selm),
            "ident": ident,
        })
    return in_maps


def run(seq, lens, cond, trace=False, **kw):
    assign, nslab, S = plan_shards(lens)
    nc = _get_program(S)
    in_maps = make_in_maps(seq, lens, cond, assign, nslab, S)
    res = run_bass_kernel_spmd(nc, in_maps, core_ids=list(range(NCORES)),
                               trace=trace, **kw)
    outs = np.zeros((B, D), np.float32)
    for c in range(NCORES):
        for bb, b in enumerate(assign[c]):
            outs[b] = res.results[c]["out"][bb]
    return outs, res


def kernel(seq, lens, cond):
    outs, _ = run(seq, lens, cond)
    return outs
